# revision 51
# baseline (speedup 1.0000x reference)
"""Trainium2 Bass kernel for nn_AuxiliaryModel_57707180589353.

Tree-conv model:
  - per-leaf 1x1 conv (scalar -> C channels) + leaf node weight
  - per-unmatched-column 1x1 conv
  - 10 levels of pairwise tree merge: Conv1d(C,C,3,'same') + BN(eval) + ReLU,
    scaled by per-node weight; every level emits a [B, C, 1024] feature chunk
  - concat all chunks along length, max-pool adjacent pairs, flatten.

Sharding: data-parallel over batch B=256 across 8 cores (32 samples/core).
All parameters are tiny and replicated.

Device layout (per core): activations live as [128, 1024] fp16 SBUF tiles:
  partition p = 16*s + c  (s = sample-in-group 0..7, c = channel 0..15),
  free dim   = spatial in "split" order: col j holds position 2j (even half,
  cols 0..511) / position 2j+1 (odd half, cols 512..1023).
The split order makes the conv taps contiguous matmuls and the final
pair-max-pool a dense tensor_tensor(max) of the two halves. x arrives
pre-permuted to split order and cast to fp16 on the host.

Conv1d(C,C,3) runs on the TensorEngine as accumulated matmuls with
block-diagonal (8 groups x 16x16) weights; BN scale is folded into the
weights host-side so the PSUM->SBUF pass is a plain bias+ReLU activation.
The shifted taps use per-node [n, hl-1] access patterns whose skipped
node-boundary column realizes the per-node 'same' zero padding directly.
Level 9 (single node) folds its node weight into the activation scale/bias,
skipping that level's DVE multiply.

The unmatched-column stage runs entirely on the DVE from a 0-stride
broadcast DMA (no matmul/PSUM), scheduled mid-kernel. Pooled outputs
accumulate in per-group SBUF band buffers for levels 0-5 and stream out
per level for 6-9 (short tail); everything is written as fp16 and upcast
on the host.
"""

import numpy as np

B = 256
L = 1024
U = 256
C = 16
LEVELS = 10
EPS = 1e-5
N_CORES = 8
BPC = B // N_CORES          # 32 samples per core
SPG = 8                     # samples per matmul group (8*16 = 128 partitions)
GROUPS = BPC // SPG         # 4
T_OUT = (L + U + LEVELS * L) // 2   # 5760
OUT_COLS = C * T_OUT        # 92160
K_FOLD = 9                  # levels >= K_FOLD fold node weight into the act

_CACHE = {}


def _build_nc():
    import concourse.bacc as bacc
    import concourse.tile as tile
    import concourse.mybir as mybir

    dt = mybir.dt
    f32 = dt.float32
    f16 = dt.float16
    Act = mybir.ActivationFunctionType
    Alu = mybir.AluOpType

    nc = bacc.Bacc("TRN2", target_bir_lowering=False, debug=False,
                   enable_asserts=False, num_devices=N_CORES)

    def din(name, shape, dtype=f16):
        return nc.dram_tensor(name, list(shape), dtype, kind="ExternalInput").ap()

    x_d = din("x", [BPC, L + U])      # split order within leaf / unm blocks
    lwB_d = din("lwB", [128, L])
    lbB_d = din("lbB", [128, L])
    uwB_d = din("uwB", [128, U])
    ubB_d = din("ubB", [128, U])
    Wc_d = din("Wc", [128, 128])
    W0_d = din("W0", [128, 128])
    W2_d = din("W2", [128, 128])
    R_d = din("R", [32, GROUPS * 128])
    b2P_d = din("b2P", [128, 1], f32)
    actW_d = din("actW", [128, 1], f32)
    actB_d = din("actB", [128, 1], f32)
    nwa_d = din("nwa", [128, 2 * L])              # levels 0-1
    nwb_d = din("nwb", [128, (K_FOLD - 2) * L])   # levels 2..K_FOLD-1
    out_d = nc.dram_tensor("out", [BPC, OUT_COLS], f16, kind="ExternalOutput").ap()

    # [4, 8, 16, 5760] view of the output: (group, sample, channel, pooled col)
    out_v = out_d.rearrange("(g s) (c t) -> g s c t", g=GROUPS, c=C)

    # pooled-column bands: (first level, n levels, out col offset)
    BANDS = [(0, 3, 640), (3, 3, 2176), (6, 1, 3712), (7, 1, 4224),
             (8, 1, 4736), (9, 1, 5248)]

    with tile.TileContext(nc) as tc:
        with (tc.tile_pool(name="consts", bufs=1) as cpool,
              tc.tile_pool(name="curp", bufs=8) as curp,
              tc.tile_pool(name="chp", bufs=8) as chp,
              tc.tile_pool(name="small", bufs=4) as smallp,
              tc.tile_pool(name="psp", bufs=4, space="PSUM") as psp):
            # ---- input + constant loads (order = DMA priority) ----
            # only the leaf block of x is staged in SBUF (the unmatched
            # columns are broadcast straight from DRAM below): 2KB-aligned
            # packets and a slightly earlier first matmul
            xb = cpool.tile([BPC, L], f16, name="xb")
            nc.sync.dma_start(out=xb, in_=x_d[:, 0:L])
            R = cpool.tile_from(R_d)
            lwB = cpool.tile_from(lwB_d)
            lbB = cpool.tile_from(lbB_d)
            Wc = cpool.tile_from(Wc_d)
            W0 = cpool.tile_from(W0_d)
            W2 = cpool.tile_from(W2_d)
            b2P = cpool.tile_from(b2P_d)
            nwa = cpool.tile_from(nwa_d)
            xus = []
            for g in range(GROUPS):
                xu = smallp.tile([128, U], f16, tag="xu", name=f"xu{g}")
                src = x_d[g * SPG:(g + 1) * SPG, L:L + U]
                nc.sync.dma_start(out=xu, in_=src.unsqueeze(1)
                                  .broadcast_to([SPG, C, U]))
                xus.append(xu)
            uwB = cpool.tile_from(uwB_d)
            ubB = cpool.tile_from(ubB_d)
            actW = cpool.tile_from(actW_d)
            actB = cpool.tile_from(actB_d)
            nwb = cpool.tile_from(nwb_d)

            def mm(out, lhsT, rhs, start, stop):
                nc.tensor.matmul(out, lhsT, rhs, start=start, stop=stop,
                                 skip_group_check=True)

            # ---- leaf stage: cur[(s,c), j] = (x[s,sig(j)]*lw'+lb') ----
            def ps_tile(name):
                return psp.tile([128, L], f32, tag="ps", name=name)

            ps_leaf = []
            for g in range(GROUPS):
                ps = ps_tile(f"psleaf{g}")
                Rg = R[:, g * 128:(g + 1) * 128]
                mm(ps[:, 0:512], Rg, xb[:, 0:512], True, True)
                mm(ps[:, 512:1024], Rg, xb[:, 512:1024], True, True)
                ps_leaf.append(ps)
            curs = []
            for g in range(GROUPS):
                cur = curp.tile([128, L], f16, tag="cur", name=f"lcur{g}")
                nc.scalar.activation(out=cur, in_=ps_leaf[g], func=Act.Copy,
                                     scale=1.0)
                nc.vector.tensor_mul(out=cur, in0=cur, in1=lwB)
                nc.vector.tensor_add(out=cur, in0=cur, in1=lbB)
                curs.append(cur)
            leaf_tiles = list(curs)

            # ---- tree levels ----
            bands = {}
            for bi, (k0b, nk, ooff) in enumerate(BANDS):
                for k in range(k0b, k0b + nk):
                    hl = 1 << k          # half node length in split cols
                    n = 512 >> k         # number of nodes at this level
                    if k < 2:
                        nwk = nwa[:, k * L:(k + 1) * L]
                    elif k < K_FOLD:
                        nwk = nwb[:, (k - 2) * L:(k - 1) * L]
                    else:
                        nwk = None       # folded into the activation
                    for g in range(GROUPS):
                        prev = curs[g]
                        ce, co = prev[:, 0:512], prev[:, 512:1024]
                        ps = ps_tile(f"ps{k}_{g}")
                        pe, po = ps[:, 0:512], ps[:, 512:1024]
                        # (dst, weights, src, is_first_write_of_its_region)
                        if k == 0:
                            # length-2 nodes: no shifted taps
                            mms = [(pe, Wc, ce, True), (po, Wc, co, True),
                                   (pe, W2, co, False), (po, W0, ce, False)]
                        else:
                            # shifted taps as per-node [n, hl-1] APs: the
                            # 'same' zero padding falls out of skipping the
                            # node-boundary column, no fixup matmuls needed
                            pe_n = pe.rearrange("p (n h) -> p n h", n=n)
                            po_n = po.rearrange("p (n h) -> p n h", n=n)
                            ce_n = ce.rearrange("p (n h) -> p n h", n=n)
                            co_n = co.rearrange("p (n h) -> p n h", n=n)
                            mms = [(pe, Wc, ce, True), (po, Wc, co, True),
                                   (po, W0, ce, False),
                                   (pe_n[:, :, 1:hl], W0,
                                    co_n[:, :, 0:hl - 1], False),
                                   (pe, W2, co, False),
                                   (po_n[:, :, 0:hl - 1], W2,
                                    ce_n[:, :, 1:hl], False)]
                        for i, (o, w, r, first) in enumerate(mms):
                            mm(o, w, r, first, i == len(mms) - 1)
                        cur = curp.tile([128, L], f16, tag="cur",
                                        name=f"cur{k}_{g}")
                        if nwk is None:
                            # node weight folded into act scale/bias (n == 1)
                            nc.scalar.activation(out=cur, in_=ps,
                                                 func=Act.Relu,
                                                 bias=actB[:, 0:1],
                                                 scale=actW[:, 0:1])
                        else:
                            nc.scalar.activation(out=cur, in_=ps,
                                                 func=Act.Relu, bias=b2P,
                                                 scale=1.0)
                            nc.vector.tensor_mul(out=cur, in0=cur, in1=nwk)
                        if k == k0b:
                            bands[g] = chp.tile([128, nk * 512], f16,
                                                tag=f"band{nk}_{bi % 2}",
                                                name=f"band{k}_{g}")
                        bslice = bands[g][:, (k - k0b) * 512:
                                          (k - k0b + 1) * 512]
                        eng = nc.sync if g % 2 == 0 else nc.gpsimd
                        if k == LEVELS - 1:
                            # last level: pair-max and stream out in halves
                            # so the final transfer is half as deep
                            for h in range(2):
                                hs = slice(h * 256, (h + 1) * 256)
                                nc.vector.tensor_tensor(
                                    out=bslice[:, hs],
                                    in0=cur[:, h * 256:(h + 1) * 256],
                                    in1=cur[:, 512 + h * 256:768 + h * 256],
                                    op=Alu.max)
                                eng.dma_start(
                                    out=out_v[g, :, :,
                                              ooff + h * 256:
                                              ooff + (h + 1) * 256],
                                    in_=bslice[:, hs])
                        else:
                            nc.vector.tensor_tensor(out=bslice,
                                                    in0=cur[:, 0:512],
                                                    in1=cur[:, 512:1024],
                                                    op=Alu.max)
                            if k == k0b + nk - 1:
                                # alternate trigger queues to avoid
                                # serializing on the Sync engine at the tail
                                eng.dma_start(
                                    out=out_v[g, :, :, ooff:ooff + nk * 512],
                                    in_=bands[g])
                        curs[g] = cur

                    if k == 0:
                        # leaf pooled output, off the startup critical path
                        for g in range(GROUPS):
                            lt = leaf_tiles[g]
                            lch = chp.tile([128, 512], f16, tag="lch",
                                           bufs=4, name=f"lch{g}")
                            nc.vector.tensor_tensor(out=lch, in0=lt[:, 0:512],
                                                    in1=lt[:, 512:1024],
                                                    op=Alu.max)
                            nc.gpsimd.dma_start(out=out_v[g, :, :, 0:512],
                                                in_=lch)
                    elif k == 1:
                        # unmatched columns (pure DVE, inputs loaded at t=0)
                        for g in range(GROUPS):
                            xu = xus[g]
                            nc.vector.tensor_mul(out=xu, in0=xu, in1=uwB)
                            nc.vector.tensor_add(out=xu, in0=xu, in1=ubB)
                            uch = smallp.tile([128, U // 2], f16, tag="uch",
                                              name=f"uch{g}")
                            nc.vector.tensor_tensor(out=uch, in0=xu[:, 0:128],
                                                    in1=xu[:, 128:256],
                                                    op=Alu.max)
                            nc.gpsimd.dma_start(out=out_v[g, :, :, 512:640],
                                                in_=uch)

    nc.compile()
    return nc


def _split_cols(a):
    """Reorder the last axis from position order to split (even|odd) order."""
    return np.concatenate([a[..., 0::2], a[..., 1::2]], axis=-1)


def _host_consts(leaf_w, leaf_b, unm_w, unm_b, conv_w, conv_b,
                 bn_gamma, bn_beta, bn_mean, bn_var, leaf_nw, internal_nw):
    f32 = np.float32
    f16 = np.float16

    s = (bn_gamma / np.sqrt(bn_var + EPS)).astype(f32)          # [C]
    b2 = ((conv_b - bn_mean) * s + bn_beta).astype(f32)

    lw = (leaf_w * leaf_nw[:, None]).astype(f32)                # [L, C]
    lb = (leaf_b * leaf_nw[:, None]).astype(f32)

    def rep_pc(v16):  # [16] -> [128, 1] (partition p = 16*s + c)
        return np.tile(np.asarray(v16, f32), SPG).reshape(128, 1)

    def bcast_cols(wLC):  # [Ncols, C] -> [128, Ncols] split order
        t = np.tile(wLC.T, (SPG, 1))                            # [128, Ncols]
        return _split_cols(t).astype(f16)

    lwB = bcast_cols(lw)
    lbB = bcast_cols(lb)
    uwB = bcast_cols(np.asarray(unm_w, f32))
    ubB = bcast_cols(np.asarray(unm_b, f32))

    def blockdiag(w16):  # 16x16 block -> [128, 128] block-diagonal
        out = np.zeros((128, 128), f32)
        for g in range(SPG):
            out[g * C:(g + 1) * C, g * C:(g + 1) * C] = w16
        return out

    # BN scale folded into the conv weights:
    # lhsT[(g,ci),(g,co)] = conv_w[co, ci, k] * s[co]
    Wk = [blockdiag((conv_w[:, :, k] * s[:, None]).T) for k in range(3)]
    Wc = Wk[1].astype(f16)
    W0 = Wk[0].astype(f16)
    W2 = Wk[2].astype(f16)

    R = np.zeros((32, GROUPS * 128), f32)
    for g in range(GROUPS):
        for sl in range(SPG):
            R[g * SPG + sl, g * 128 + sl * C:g * 128 + (sl + 1) * C] = 1.0
    R = R.astype(f16)

    b2P = rep_pc(b2)

    # node-weight vectors for the non-folded levels, split order; the last
    # level's single node weight goes into the activation scale/bias
    nwB = np.zeros((128, K_FOLD * L), f32)
    off = 0
    w_last = None
    for k in range(LEVELS):
        n = L >> (k + 1)
        w = np.asarray(internal_nw[off:off + n], f32)
        off += n
        if k < K_FOLD:
            expand = np.repeat(w, 1 << (k + 1))      # [1024] position order
            nwB[:, k * L:(k + 1) * L] = _split_cols(expand)[None, :]
        else:
            w_last = float(w[0])
    nwB = nwB.astype(f16)

    actW = np.full((128, 1), w_last, f32)
    actB = (w_last * b2P).astype(f32)

    return {
        "lwB": lwB, "lbB": lbB, "uwB": uwB, "ubB": ubB,
        "Wc": Wc, "W0": W0, "W2": W2, "R": R,
        "b2P": b2P, "actW": actW, "actB": actB,
        "nwa": np.ascontiguousarray(nwB[:, 0:2 * L]),
        "nwb": np.ascontiguousarray(nwB[:, 2 * L:]),
    }


def _make_in_maps(inputs):
    consts = _host_consts(
        np.asarray(inputs["leaf_w"]), np.asarray(inputs["leaf_b"]),
        np.asarray(inputs["unm_w"]), np.asarray(inputs["unm_b"]),
        np.asarray(inputs["conv_w"]), np.asarray(inputs["conv_b"]),
        np.asarray(inputs["bn_gamma"]), np.asarray(inputs["bn_beta"]),
        np.asarray(inputs["bn_mean"]), np.asarray(inputs["bn_var"]),
        np.asarray(inputs["leaf_nw"]), np.asarray(inputs["internal_nw"]))
    x = np.asarray(inputs["x"], np.float32)
    # split order within the leaf block and within the unmatched block
    xs = np.concatenate([x[:, 0:L:2], x[:, 1:L:2],
                         x[:, L:L + U:2], x[:, L + 1:L + U:2]],
                        axis=1).astype(np.float16)
    in_maps = []
    for c in range(N_CORES):
        m = dict(consts)
        m["x"] = np.ascontiguousarray(xs[c * BPC:(c + 1) * BPC])
        in_maps.append(m)
    return in_maps


def kernel(x, leaf_w, leaf_b, unm_w, unm_b, conv_w, conv_b,
           bn_gamma, bn_beta, bn_mean, bn_var, leaf_nw, internal_nw):
    from concourse.bass_utils import run_bass_kernel_spmd

    if "nc" not in _CACHE:
        _CACHE["nc"] = _build_nc()
    nc = _CACHE["nc"]

    in_maps = _make_in_maps({
        "x": x, "leaf_w": leaf_w, "leaf_b": leaf_b, "unm_w": unm_w,
        "unm_b": unm_b, "conv_w": conv_w, "conv_b": conv_b,
        "bn_gamma": bn_gamma, "bn_beta": bn_beta, "bn_mean": bn_mean,
        "bn_var": bn_var, "leaf_nw": leaf_nw, "internal_nw": internal_nw})

    res = run_bass_kernel_spmd(nc, in_maps, core_ids=list(range(N_CORES)))
    out = np.concatenate([r["out"] for r in res.results], axis=0)
    return out.astype(np.float32)


# revision 52
# speedup vs baseline: 1.0216x; 1.0216x over previous
"""Trainium2 Bass kernel for nn_AuxiliaryModel_57707180589353.

Tree-conv model:
  - per-leaf 1x1 conv (scalar -> C channels) + leaf node weight
  - per-unmatched-column 1x1 conv
  - 10 levels of pairwise tree merge: Conv1d(C,C,3,'same') + BN(eval) + ReLU,
    scaled by per-node weight; every level emits a [B, C, 1024] feature chunk
  - concat all chunks along length, max-pool adjacent pairs, flatten.

Sharding: data-parallel over batch B=256 across 8 cores (32 samples/core).
All parameters are tiny and replicated.

Device layout (per core): activations live as [128, 1024] fp16 SBUF tiles:
  partition p = 16*s + c  (s = sample-in-group 0..7, c = channel 0..15),
  free dim   = spatial in "split" order: col j holds position 2j (even half,
  cols 0..511) / position 2j+1 (odd half, cols 512..1023).
The split order makes the conv taps contiguous matmuls and the final
pair-max-pool a dense tensor_tensor(max) of the two halves. x arrives
pre-permuted to split order and cast to fp16 on the host.

Conv1d(C,C,3) runs on the TensorEngine as accumulated matmuls with
block-diagonal (8 groups x 16x16) weights; BN scale is folded into the
weights host-side so the PSUM->SBUF pass is a plain bias+ReLU activation.
The shifted taps use per-node [n, hl-1] access patterns whose skipped
node-boundary column realizes the per-node 'same' zero padding directly.
Level 9 (single node) folds its node weight into the activation scale/bias,
skipping that level's DVE multiply.

The unmatched-column stage runs entirely on the DVE from a 0-stride
broadcast DMA (no matmul/PSUM), scheduled mid-kernel. Pooled outputs
accumulate in per-group SBUF band buffers for levels 0-5 and stream out
per level for 6-9 (short tail); everything is written as fp16 and upcast
on the host.
"""

import numpy as np

B = 256
L = 1024
U = 256
C = 16
LEVELS = 10
EPS = 1e-5
N_CORES = 8
BPC = B // N_CORES          # 32 samples per core
SPG = 8                     # samples per matmul group (8*16 = 128 partitions)
GROUPS = BPC // SPG         # 4
T_OUT = (L + U + LEVELS * L) // 2   # 5760
OUT_COLS = C * T_OUT        # 92160
K_FOLD = 9                  # levels >= K_FOLD fold node weight into the act

_CACHE = {}


def _build_nc():
    import concourse.bacc as bacc
    import concourse.tile as tile
    import concourse.mybir as mybir

    dt = mybir.dt
    f32 = dt.float32
    f16 = dt.float16
    Act = mybir.ActivationFunctionType
    Alu = mybir.AluOpType

    nc = bacc.Bacc("TRN2", target_bir_lowering=False, debug=False,
                   enable_asserts=False, num_devices=N_CORES)

    def din(name, shape, dtype=f16):
        return nc.dram_tensor(name, list(shape), dtype, kind="ExternalInput").ap()

    x_d = din("x", [BPC, L + U])      # split order within leaf / unm blocks
    lwB_d = din("lwB", [128, L])
    lbB_d = din("lbB", [128, L])
    uwB_d = din("uwB", [128, U])
    ubB_d = din("ubB", [128, U])
    Wc_d = din("Wc", [128, 128])
    W0_d = din("W0", [128, 128])
    W2_d = din("W2", [128, 128])
    R_d = din("R", [32, GROUPS * 128])
    b2P_d = din("b2P", [128, 1], f32)
    actW_d = din("actW", [128, 1], f32)
    actB_d = din("actB", [128, 1], f32)
    nwa_d = din("nwa", [128, 2 * L])              # levels 0-1
    nwb_d = din("nwb", [128, (K_FOLD - 2) * L])   # levels 2..K_FOLD-1
    out_d = nc.dram_tensor("out", [BPC, OUT_COLS], f16, kind="ExternalOutput").ap()

    # [4, 8, 16, 5760] view of the output: (group, sample, channel, pooled col)
    out_v = out_d.rearrange("(g s) (c t) -> g s c t", g=GROUPS, c=C)

    # pooled-column bands: (first level, n levels, out col offset)
    BANDS = [(0, 3, 640), (3, 3, 2176), (6, 1, 3712), (7, 1, 4224),
             (8, 1, 4736), (9, 1, 5248)]

    with tile.TileContext(nc) as tc:
        with (tc.tile_pool(name="consts", bufs=1) as cpool,
              tc.tile_pool(name="curp", bufs=8) as curp,
              tc.tile_pool(name="chp", bufs=8) as chp,
              tc.tile_pool(name="small", bufs=4) as smallp,
              tc.tile_pool(name="psp", bufs=4, space="PSUM") as psp):
            # ---- input + constant loads (order = DMA priority) ----
            # only the leaf block of x is staged in SBUF (the unmatched
            # columns are broadcast straight from DRAM below): 2KB-aligned
            # packets and a slightly earlier first matmul
            xb = cpool.tile([BPC, L], f16, name="xb")
            nc.sync.dma_start(out=xb, in_=x_d[:, 0:L])
            R = cpool.tile_from(R_d)
            lwB = cpool.tile_from(lwB_d)
            lbB = cpool.tile_from(lbB_d)
            Wc = cpool.tile_from(Wc_d)
            W0 = cpool.tile_from(W0_d)
            W2 = cpool.tile_from(W2_d)
            b2P = cpool.tile_from(b2P_d)
            nwa = cpool.tile_from(nwa_d)
            xus = []
            for g in range(GROUPS):
                xu = smallp.tile([128, U], f16, tag="xu", name=f"xu{g}")
                src = x_d[g * SPG:(g + 1) * SPG, L:L + U]
                nc.sync.dma_start(out=xu, in_=src.unsqueeze(1)
                                  .broadcast_to([SPG, C, U]))
                xus.append(xu)
            uwB = cpool.tile_from(uwB_d)
            ubB = cpool.tile_from(ubB_d)
            actW = cpool.tile_from(actW_d)
            actB = cpool.tile_from(actB_d)
            nwb = cpool.tile_from(nwb_d)

            def mm(out, lhsT, rhs, start, stop):
                nc.tensor.matmul(out, lhsT, rhs, start=start, stop=stop,
                                 skip_group_check=True)

            # ---- leaf stage: cur[(s,c), j] = (x[s,sig(j)]*lw'+lb') ----
            def ps_tile(name):
                return psp.tile([128, L], f32, tag="ps", name=name)

            ps_leaf = []
            for g in range(GROUPS):
                ps = ps_tile(f"psleaf{g}")
                Rg = R[:, g * 128:(g + 1) * 128]
                mm(ps[:, 0:512], Rg, xb[:, 0:512], True, True)
                mm(ps[:, 512:1024], Rg, xb[:, 512:1024], True, True)
                ps_leaf.append(ps)
            curs = []
            for g in range(GROUPS):
                cur = curp.tile([128, L], f16, tag="cur", name=f"lcur{g}")
                nc.scalar.activation(out=cur, in_=ps_leaf[g], func=Act.Copy,
                                     scale=1.0)
                nc.vector.tensor_mul(out=cur, in0=cur, in1=lwB)
                nc.vector.tensor_add(out=cur, in0=cur, in1=lbB)
                curs.append(cur)
            leaf_tiles = list(curs)

            # ---- tree levels ----
            bands = {}
            for bi, (k0b, nk, ooff) in enumerate(BANDS):
                for k in range(k0b, k0b + nk):
                    hl = 1 << k          # half node length in split cols
                    n = 512 >> k         # number of nodes at this level
                    if k < 2:
                        nwk = nwa[:, k * L:(k + 1) * L]
                    elif k < K_FOLD:
                        nwk = nwb[:, (k - 2) * L:(k - 1) * L]
                    else:
                        nwk = None       # folded into the activation
                    for g in range(GROUPS):
                        prev = curs[g]
                        ce, co = prev[:, 0:512], prev[:, 512:1024]
                        ps = ps_tile(f"ps{k}_{g}")
                        pe, po = ps[:, 0:512], ps[:, 512:1024]
                        # (dst, weights, src, is_first_write_of_its_region)
                        if k == 0:
                            # length-2 nodes: no shifted taps
                            mms = [(pe, Wc, ce, True), (po, Wc, co, True),
                                   (pe, W2, co, False), (po, W0, ce, False)]
                        else:
                            # shifted taps as per-node [n, hl-1] APs: the
                            # 'same' zero padding falls out of skipping the
                            # node-boundary column, no fixup matmuls needed
                            pe_n = pe.rearrange("p (n h) -> p n h", n=n)
                            po_n = po.rearrange("p (n h) -> p n h", n=n)
                            ce_n = ce.rearrange("p (n h) -> p n h", n=n)
                            co_n = co.rearrange("p (n h) -> p n h", n=n)
                            mms = [(pe, Wc, ce, True), (po, Wc, co, True),
                                   (po, W0, ce, False),
                                   (pe_n[:, :, 1:hl], W0,
                                    co_n[:, :, 0:hl - 1], False),
                                   (pe, W2, co, False),
                                   (po_n[:, :, 0:hl - 1], W2,
                                    ce_n[:, :, 1:hl], False)]
                        for i, (o, w, r, first) in enumerate(mms):
                            mm(o, w, r, first, i == len(mms) - 1)
                        cur = curp.tile([128, L], f16, tag="cur",
                                        name=f"cur{k}_{g}")
                        if nwk is None:
                            # node weight folded into act scale/bias (n == 1)
                            nc.scalar.activation(out=cur, in_=ps,
                                                 func=Act.Relu,
                                                 bias=actB[:, 0:1],
                                                 scale=actW[:, 0:1])
                        else:
                            nc.scalar.activation(out=cur, in_=ps,
                                                 func=Act.Relu, bias=b2P,
                                                 scale=1.0)
                            nc.vector.tensor_mul(out=cur, in0=cur, in1=nwk)
                        if k == k0b:
                            bands[g] = chp.tile([128, nk * 512], f16,
                                                tag=f"band{nk}_{bi % 2}",
                                                name=f"band{k}_{g}")
                        bslice = bands[g][:, (k - k0b) * 512:
                                          (k - k0b + 1) * 512]
                        nc.vector.tensor_tensor(out=bslice, in0=cur[:, 0:512],
                                                in1=cur[:, 512:1024],
                                                op=Alu.max)
                        if k == k0b + nk - 1:
                            # alternate trigger queues to avoid serializing
                            # on the Sync engine at the tail
                            eng = nc.sync if g % 2 == 0 else nc.gpsimd
                            eng.dma_start(
                                out=out_v[g, :, :, ooff:ooff + nk * 512],
                                in_=bands[g])
                        curs[g] = cur

                    if k == 0:
                        # leaf pooled output, off the startup critical path
                        for g in range(GROUPS):
                            lt = leaf_tiles[g]
                            lch = chp.tile([128, 512], f16, tag="lch",
                                           bufs=4, name=f"lch{g}")
                            nc.vector.tensor_tensor(out=lch, in0=lt[:, 0:512],
                                                    in1=lt[:, 512:1024],
                                                    op=Alu.max)
                            nc.gpsimd.dma_start(out=out_v[g, :, :, 0:512],
                                                in_=lch)
                    elif k == 1:
                        # unmatched columns (pure DVE, inputs loaded at t=0)
                        for g in range(GROUPS):
                            xu = xus[g]
                            nc.vector.tensor_mul(out=xu, in0=xu, in1=uwB)
                            nc.vector.tensor_add(out=xu, in0=xu, in1=ubB)
                            uch = smallp.tile([128, U // 2], f16, tag="uch",
                                              name=f"uch{g}")
                            nc.vector.tensor_tensor(out=uch, in0=xu[:, 0:128],
                                                    in1=xu[:, 128:256],
                                                    op=Alu.max)
                            nc.gpsimd.dma_start(out=out_v[g, :, :, 512:640],
                                                in_=uch)

    nc.compile()
    return nc


def _split_cols(a):
    """Reorder the last axis from position order to split (even|odd) order."""
    return np.concatenate([a[..., 0::2], a[..., 1::2]], axis=-1)


def _host_consts(leaf_w, leaf_b, unm_w, unm_b, conv_w, conv_b,
                 bn_gamma, bn_beta, bn_mean, bn_var, leaf_nw, internal_nw):
    f32 = np.float32
    f16 = np.float16

    s = (bn_gamma / np.sqrt(bn_var + EPS)).astype(f32)          # [C]
    b2 = ((conv_b - bn_mean) * s + bn_beta).astype(f32)

    lw = (leaf_w * leaf_nw[:, None]).astype(f32)                # [L, C]
    lb = (leaf_b * leaf_nw[:, None]).astype(f32)

    def rep_pc(v16):  # [16] -> [128, 1] (partition p = 16*s + c)
        return np.tile(np.asarray(v16, f32), SPG).reshape(128, 1)

    def bcast_cols(wLC):  # [Ncols, C] -> [128, Ncols] split order
        t = np.tile(wLC.T, (SPG, 1))                            # [128, Ncols]
        return _split_cols(t).astype(f16)

    lwB = bcast_cols(lw)
    lbB = bcast_cols(lb)
    uwB = bcast_cols(np.asarray(unm_w, f32))
    ubB = bcast_cols(np.asarray(unm_b, f32))

    def blockdiag(w16):  # 16x16 block -> [128, 128] block-diagonal
        out = np.zeros((128, 128), f32)
        for g in range(SPG):
            out[g * C:(g + 1) * C, g * C:(g + 1) * C] = w16
        return out

    # BN scale folded into the conv weights:
    # lhsT[(g,ci),(g,co)] = conv_w[co, ci, k] * s[co]
    Wk = [blockdiag((conv_w[:, :, k] * s[:, None]).T) for k in range(3)]
    Wc = Wk[1].astype(f16)
    W0 = Wk[0].astype(f16)
    W2 = Wk[2].astype(f16)

    R = np.zeros((32, GROUPS * 128), f32)
    for g in range(GROUPS):
        for sl in range(SPG):
            R[g * SPG + sl, g * 128 + sl * C:g * 128 + (sl + 1) * C] = 1.0
    R = R.astype(f16)

    b2P = rep_pc(b2)

    # node-weight vectors for the non-folded levels, split order; the last
    # level's single node weight goes into the activation scale/bias
    nwB = np.zeros((128, K_FOLD * L), f32)
    off = 0
    w_last = None
    for k in range(LEVELS):
        n = L >> (k + 1)
        w = np.asarray(internal_nw[off:off + n], f32)
        off += n
        if k < K_FOLD:
            expand = np.repeat(w, 1 << (k + 1))      # [1024] position order
            nwB[:, k * L:(k + 1) * L] = _split_cols(expand)[None, :]
        else:
            w_last = float(w[0])
    nwB = nwB.astype(f16)

    actW = np.full((128, 1), w_last, f32)
    actB = (w_last * b2P).astype(f32)

    return {
        "lwB": lwB, "lbB": lbB, "uwB": uwB, "ubB": ubB,
        "Wc": Wc, "W0": W0, "W2": W2, "R": R,
        "b2P": b2P, "actW": actW, "actB": actB,
        "nwa": np.ascontiguousarray(nwB[:, 0:2 * L]),
        "nwb": np.ascontiguousarray(nwB[:, 2 * L:]),
    }


def _make_in_maps(inputs):
    consts = _host_consts(
        np.asarray(inputs["leaf_w"]), np.asarray(inputs["leaf_b"]),
        np.asarray(inputs["unm_w"]), np.asarray(inputs["unm_b"]),
        np.asarray(inputs["conv_w"]), np.asarray(inputs["conv_b"]),
        np.asarray(inputs["bn_gamma"]), np.asarray(inputs["bn_beta"]),
        np.asarray(inputs["bn_mean"]), np.asarray(inputs["bn_var"]),
        np.asarray(inputs["leaf_nw"]), np.asarray(inputs["internal_nw"]))
    x = np.asarray(inputs["x"], np.float32)
    # split order within the leaf block and within the unmatched block
    xs = np.concatenate([x[:, 0:L:2], x[:, 1:L:2],
                         x[:, L:L + U:2], x[:, L + 1:L + U:2]],
                        axis=1).astype(np.float16)
    in_maps = []
    for c in range(N_CORES):
        m = dict(consts)
        m["x"] = np.ascontiguousarray(xs[c * BPC:(c + 1) * BPC])
        in_maps.append(m)
    return in_maps


def kernel(x, leaf_w, leaf_b, unm_w, unm_b, conv_w, conv_b,
           bn_gamma, bn_beta, bn_mean, bn_var, leaf_nw, internal_nw):
    from concourse.bass_utils import run_bass_kernel_spmd

    if "nc" not in _CACHE:
        _CACHE["nc"] = _build_nc()
    nc = _CACHE["nc"]

    in_maps = _make_in_maps({
        "x": x, "leaf_w": leaf_w, "leaf_b": leaf_b, "unm_w": unm_w,
        "unm_b": unm_b, "conv_w": conv_w, "conv_b": conv_b,
        "bn_gamma": bn_gamma, "bn_beta": bn_beta, "bn_mean": bn_mean,
        "bn_var": bn_var, "leaf_nw": leaf_nw, "internal_nw": internal_nw})

    res = run_bass_kernel_spmd(nc, in_maps, core_ids=list(range(N_CORES)))
    out = np.concatenate([r["out"] for r in res.results], axis=0)
    return out.astype(np.float32)


# revision 53
# speedup vs baseline: 1.0275x; 1.0057x over previous
"""Trainium2 Bass kernel for nn_AuxiliaryModel_57707180589353.

Tree-conv model:
  - per-leaf 1x1 conv (scalar -> C channels) + leaf node weight
  - per-unmatched-column 1x1 conv
  - 10 levels of pairwise tree merge: Conv1d(C,C,3,'same') + BN(eval) + ReLU,
    scaled by per-node weight; every level emits a [B, C, 1024] feature chunk
  - concat all chunks along length, max-pool adjacent pairs, flatten.

Sharding: data-parallel over batch B=256 across 8 cores (32 samples/core).
All parameters are tiny and replicated.

Device layout (per core): activations live as [128, 1024] fp16 SBUF tiles:
  partition p = 16*s + c  (s = sample-in-group 0..7, c = channel 0..15),
  free dim   = spatial in "split" order: col j holds position 2j (even half,
  cols 0..511) / position 2j+1 (odd half, cols 512..1023).
The split order makes the conv taps contiguous matmuls and the final
pair-max-pool a dense tensor_tensor(max) of the two halves. x arrives
pre-permuted to split order and cast to fp16 on the host.

Conv1d(C,C,3) runs on the TensorEngine as accumulated matmuls with
block-diagonal (8 groups x 16x16) weights; BN scale is folded into the
weights host-side so the PSUM->SBUF pass is a plain bias+ReLU activation.
The shifted taps use per-node [n, hl-1] access patterns whose skipped
node-boundary column realizes the per-node 'same' zero padding directly.
Level 9 (single node) folds its node weight into the activation scale/bias,
skipping that level's DVE multiply.

The unmatched-column stage runs entirely on the DVE from a 0-stride
broadcast DMA (no matmul/PSUM), scheduled mid-kernel. Pooled outputs
accumulate in per-group SBUF band buffers for levels 0-5 and stream out
per level for 6-9 (short tail); everything is written as fp16 and upcast
on the host.
"""

import numpy as np

B = 256
L = 1024
U = 256
C = 16
LEVELS = 10
EPS = 1e-5
N_CORES = 8
BPC = B // N_CORES          # 32 samples per core
SPG = 8                     # samples per matmul group (8*16 = 128 partitions)
GROUPS = BPC // SPG         # 4
T_OUT = (L + U + LEVELS * L) // 2   # 5760
OUT_COLS = C * T_OUT        # 92160
K_FOLD = 9                  # levels >= K_FOLD fold node weight into the act

_CACHE = {}


def _build_nc():
    import concourse.bacc as bacc
    import concourse.tile as tile
    import concourse.mybir as mybir

    dt = mybir.dt
    f32 = dt.float32
    f16 = dt.float16
    Act = mybir.ActivationFunctionType
    Alu = mybir.AluOpType

    nc = bacc.Bacc("TRN2", target_bir_lowering=False, debug=False,
                   enable_asserts=False, num_devices=N_CORES)

    def din(name, shape, dtype=f16):
        return nc.dram_tensor(name, list(shape), dtype, kind="ExternalInput").ap()

    x_d = din("x", [BPC, L + U])      # split order within leaf / unm blocks
    lwB_d = din("lwB", [128, L])
    lbB_d = din("lbB", [128, L])
    uwB_d = din("uwB", [128, U])
    ubB_d = din("ubB", [128, U])
    Wc_d = din("Wc", [128, 128])
    W0_d = din("W0", [128, 128])
    W2_d = din("W2", [128, 128])
    R_d = din("R", [32, GROUPS * 128])
    b2P_d = din("b2P", [128, 1], f32)
    actW_d = din("actW", [128, 1], f32)
    actB_d = din("actB", [128, 1], f32)
    nwa_d = din("nwa", [128, 2 * L])              # levels 0-1
    nwb_d = din("nwb", [128, (K_FOLD - 2) * L])   # levels 2..K_FOLD-1
    out_d = nc.dram_tensor("out", [BPC, OUT_COLS], f16, kind="ExternalOutput").ap()

    # [4, 8, 16, 5760] view of the output: (group, sample, channel, pooled col)
    out_v = out_d.rearrange("(g s) (c t) -> g s c t", g=GROUPS, c=C)

    # pooled-column bands: (first level, n levels, out col offset)
    BANDS = [(0, 3, 640), (3, 3, 2176), (6, 1, 3712), (7, 1, 4224),
             (8, 1, 4736), (9, 1, 5248)]

    with tile.TileContext(nc) as tc:
        with (tc.tile_pool(name="consts", bufs=1) as cpool,
              tc.tile_pool(name="curp", bufs=8) as curp,
              tc.tile_pool(name="chp", bufs=8) as chp,
              tc.tile_pool(name="small", bufs=4) as smallp,
              tc.tile_pool(name="psp", bufs=4, space="PSUM") as psp):
            # ---- input + constant loads (order = DMA priority) ----
            xb = cpool.tile_from(x_d)
            R = cpool.tile_from(R_d)
            lwB = cpool.tile_from(lwB_d)
            lbB = cpool.tile_from(lbB_d)
            Wc = cpool.tile_from(Wc_d)
            W0 = cpool.tile_from(W0_d)
            W2 = cpool.tile_from(W2_d)
            b2P = cpool.tile_from(b2P_d)
            nwa = cpool.tile_from(nwa_d)
            xus = []
            for g in range(GROUPS):
                xu = smallp.tile([128, U], f16, tag="xu", name=f"xu{g}")
                src = x_d[g * SPG:(g + 1) * SPG, L:L + U]
                nc.sync.dma_start(out=xu, in_=src.unsqueeze(1)
                                  .broadcast_to([SPG, C, U]))
                xus.append(xu)
            uwB = cpool.tile_from(uwB_d)
            ubB = cpool.tile_from(ubB_d)
            actW = cpool.tile_from(actW_d)
            actB = cpool.tile_from(actB_d)
            nwb = cpool.tile_from(nwb_d)

            def mm(out, lhsT, rhs, start, stop):
                nc.tensor.matmul(out, lhsT, rhs, start=start, stop=stop,
                                 skip_group_check=True)

            # ---- leaf stage: cur[(s,c), j] = (x[s,sig(j)]*lw'+lb') ----
            def ps_tile(name):
                return psp.tile([128, L], f32, tag="ps", name=name)

            ps_leaf = []
            for g in range(GROUPS):
                ps = ps_tile(f"psleaf{g}")
                Rg = R[:, g * 128:(g + 1) * 128]
                mm(ps[:, 0:512], Rg, xb[:, 0:512], True, True)
                mm(ps[:, 512:1024], Rg, xb[:, 512:1024], True, True)
                ps_leaf.append(ps)
            curs = []
            for g in range(GROUPS):
                cur = curp.tile([128, L], f16, tag="cur", name=f"lcur{g}")
                nc.scalar.activation(out=cur, in_=ps_leaf[g], func=Act.Copy,
                                     scale=1.0)
                nc.vector.tensor_mul(out=cur, in0=cur, in1=lwB)
                nc.vector.tensor_add(out=cur, in0=cur, in1=lbB)
                curs.append(cur)
            leaf_tiles = list(curs)

            # ---- tree levels ----
            bands = {}
            for bi, (k0b, nk, ooff) in enumerate(BANDS):
                for k in range(k0b, k0b + nk):
                    hl = 1 << k          # half node length in split cols
                    n = 512 >> k         # number of nodes at this level
                    if k < 2:
                        nwk = nwa[:, k * L:(k + 1) * L]
                    elif k < K_FOLD:
                        nwk = nwb[:, (k - 2) * L:(k - 1) * L]
                    else:
                        nwk = None       # folded into the activation
                    for g in range(GROUPS):
                        prev = curs[g]
                        ce, co = prev[:, 0:512], prev[:, 512:1024]
                        ps = ps_tile(f"ps{k}_{g}")
                        pe, po = ps[:, 0:512], ps[:, 512:1024]
                        # (dst, weights, src, is_first_write_of_its_region)
                        if k == 0:
                            # length-2 nodes: no shifted taps
                            mms = [(pe, Wc, ce, True), (po, Wc, co, True),
                                   (pe, W2, co, False), (po, W0, ce, False)]
                        else:
                            # shifted taps as per-node [n, hl-1] APs: the
                            # 'same' zero padding falls out of skipping the
                            # node-boundary column, no fixup matmuls needed
                            pe_n = pe.rearrange("p (n h) -> p n h", n=n)
                            po_n = po.rearrange("p (n h) -> p n h", n=n)
                            ce_n = ce.rearrange("p (n h) -> p n h", n=n)
                            co_n = co.rearrange("p (n h) -> p n h", n=n)
                            mms = [(pe, Wc, ce, True), (po, Wc, co, True),
                                   (po, W0, ce, False),
                                   (pe_n[:, :, 1:hl], W0,
                                    co_n[:, :, 0:hl - 1], False),
                                   (pe, W2, co, False),
                                   (po_n[:, :, 0:hl - 1], W2,
                                    ce_n[:, :, 1:hl], False)]
                        for i, (o, w, r, first) in enumerate(mms):
                            mm(o, w, r, first, i == len(mms) - 1)
                        cur = curp.tile([128, L], f16, tag="cur",
                                        name=f"cur{k}_{g}")
                        if nwk is None:
                            # node weight folded into act scale/bias (n == 1)
                            nc.scalar.activation(out=cur, in_=ps,
                                                 func=Act.Relu,
                                                 bias=actB[:, 0:1],
                                                 scale=actW[:, 0:1])
                        else:
                            nc.scalar.activation(out=cur, in_=ps,
                                                 func=Act.Relu, bias=b2P,
                                                 scale=1.0)
                            nc.vector.tensor_mul(out=cur, in0=cur, in1=nwk)
                        if k == k0b:
                            bands[g] = chp.tile([128, nk * 512], f16,
                                                tag=f"band{nk}_{bi % 2}",
                                                name=f"band{k}_{g}")
                        bslice = bands[g][:, (k - k0b) * 512:
                                          (k - k0b + 1) * 512]
                        nc.vector.tensor_tensor(out=bslice, in0=cur[:, 0:512],
                                                in1=cur[:, 512:1024],
                                                op=Alu.max)
                        if k == k0b + nk - 1:
                            # alternate trigger queues to avoid serializing
                            # on the Sync engine at the tail
                            eng = nc.sync if g % 2 == 0 else nc.gpsimd
                            eng.dma_start(
                                out=out_v[g, :, :, ooff:ooff + nk * 512],
                                in_=bands[g])
                        curs[g] = cur

                    if k == 0:
                        # leaf pooled output, off the startup critical path
                        for g in range(GROUPS):
                            lt = leaf_tiles[g]
                            lch = chp.tile([128, 512], f16, tag="lch",
                                           bufs=4, name=f"lch{g}")
                            nc.vector.tensor_tensor(out=lch, in0=lt[:, 0:512],
                                                    in1=lt[:, 512:1024],
                                                    op=Alu.max)
                            nc.gpsimd.dma_start(out=out_v[g, :, :, 0:512],
                                                in_=lch)
                    elif k == 1:
                        # unmatched columns (pure DVE, inputs loaded at t=0)
                        for g in range(GROUPS):
                            xu = xus[g]
                            nc.vector.tensor_mul(out=xu, in0=xu, in1=uwB)
                            nc.vector.tensor_add(out=xu, in0=xu, in1=ubB)
                            uch = smallp.tile([128, U // 2], f16, tag="uch",
                                              name=f"uch{g}")
                            nc.vector.tensor_tensor(out=uch, in0=xu[:, 0:128],
                                                    in1=xu[:, 128:256],
                                                    op=Alu.max)
                            nc.gpsimd.dma_start(out=out_v[g, :, :, 512:640],
                                                in_=uch)

    nc.compile()
    return nc


def _split_cols(a):
    """Reorder the last axis from position order to split (even|odd) order."""
    return np.concatenate([a[..., 0::2], a[..., 1::2]], axis=-1)


def _host_consts(leaf_w, leaf_b, unm_w, unm_b, conv_w, conv_b,
                 bn_gamma, bn_beta, bn_mean, bn_var, leaf_nw, internal_nw):
    f32 = np.float32
    f16 = np.float16

    s = (bn_gamma / np.sqrt(bn_var + EPS)).astype(f32)          # [C]
    b2 = ((conv_b - bn_mean) * s + bn_beta).astype(f32)

    lw = (leaf_w * leaf_nw[:, None]).astype(f32)                # [L, C]
    lb = (leaf_b * leaf_nw[:, None]).astype(f32)

    def rep_pc(v16):  # [16] -> [128, 1] (partition p = 16*s + c)
        return np.tile(np.asarray(v16, f32), SPG).reshape(128, 1)

    def bcast_cols(wLC):  # [Ncols, C] -> [128, Ncols] split order
        t = np.tile(wLC.T, (SPG, 1))                            # [128, Ncols]
        return _split_cols(t).astype(f16)

    lwB = bcast_cols(lw)
    lbB = bcast_cols(lb)
    uwB = bcast_cols(np.asarray(unm_w, f32))
    ubB = bcast_cols(np.asarray(unm_b, f32))

    def blockdiag(w16):  # 16x16 block -> [128, 128] block-diagonal
        out = np.zeros((128, 128), f32)
        for g in range(SPG):
            out[g * C:(g + 1) * C, g * C:(g + 1) * C] = w16
        return out

    # BN scale folded into the conv weights:
    # lhsT[(g,ci),(g,co)] = conv_w[co, ci, k] * s[co]
    Wk = [blockdiag((conv_w[:, :, k] * s[:, None]).T) for k in range(3)]
    Wc = Wk[1].astype(f16)
    W0 = Wk[0].astype(f16)
    W2 = Wk[2].astype(f16)

    R = np.zeros((32, GROUPS * 128), f32)
    for g in range(GROUPS):
        for sl in range(SPG):
            R[g * SPG + sl, g * 128 + sl * C:g * 128 + (sl + 1) * C] = 1.0
    R = R.astype(f16)

    b2P = rep_pc(b2)

    # node-weight vectors for the non-folded levels, split order; the last
    # level's single node weight goes into the activation scale/bias
    nwB = np.zeros((128, K_FOLD * L), f32)
    off = 0
    w_last = None
    for k in range(LEVELS):
        n = L >> (k + 1)
        w = np.asarray(internal_nw[off:off + n], f32)
        off += n
        if k < K_FOLD:
            expand = np.repeat(w, 1 << (k + 1))      # [1024] position order
            nwB[:, k * L:(k + 1) * L] = _split_cols(expand)[None, :]
        else:
            w_last = float(w[0])
    nwB = nwB.astype(f16)

    actW = np.full((128, 1), w_last, f32)
    actB = (w_last * b2P).astype(f32)

    return {
        "lwB": lwB, "lbB": lbB, "uwB": uwB, "ubB": ubB,
        "Wc": Wc, "W0": W0, "W2": W2, "R": R,
        "b2P": b2P, "actW": actW, "actB": actB,
        "nwa": np.ascontiguousarray(nwB[:, 0:2 * L]),
        "nwb": np.ascontiguousarray(nwB[:, 2 * L:]),
    }


def _make_in_maps(inputs):
    consts = _host_consts(
        np.asarray(inputs["leaf_w"]), np.asarray(inputs["leaf_b"]),
        np.asarray(inputs["unm_w"]), np.asarray(inputs["unm_b"]),
        np.asarray(inputs["conv_w"]), np.asarray(inputs["conv_b"]),
        np.asarray(inputs["bn_gamma"]), np.asarray(inputs["bn_beta"]),
        np.asarray(inputs["bn_mean"]), np.asarray(inputs["bn_var"]),
        np.asarray(inputs["leaf_nw"]), np.asarray(inputs["internal_nw"]))
    x = np.asarray(inputs["x"], np.float32)
    # split order within the leaf block and within the unmatched block
    xs = np.concatenate([x[:, 0:L:2], x[:, 1:L:2],
                         x[:, L:L + U:2], x[:, L + 1:L + U:2]],
                        axis=1).astype(np.float16)
    in_maps = []
    for c in range(N_CORES):
        m = dict(consts)
        m["x"] = np.ascontiguousarray(xs[c * BPC:(c + 1) * BPC])
        in_maps.append(m)
    return in_maps


def kernel(x, leaf_w, leaf_b, unm_w, unm_b, conv_w, conv_b,
           bn_gamma, bn_beta, bn_mean, bn_var, leaf_nw, internal_nw):
    from concourse.bass_utils import run_bass_kernel_spmd

    if "nc" not in _CACHE:
        _CACHE["nc"] = _build_nc()
    nc = _CACHE["nc"]

    in_maps = _make_in_maps({
        "x": x, "leaf_w": leaf_w, "leaf_b": leaf_b, "unm_w": unm_w,
        "unm_b": unm_b, "conv_w": conv_w, "conv_b": conv_b,
        "bn_gamma": bn_gamma, "bn_beta": bn_beta, "bn_mean": bn_mean,
        "bn_var": bn_var, "leaf_nw": leaf_nw, "internal_nw": internal_nw})

    res = run_bass_kernel_spmd(nc, in_maps, core_ids=list(range(N_CORES)))
    out = np.concatenate([r["out"] for r in res.results], axis=0)
    return out.astype(np.float32)
